# revision 8
# baseline (speedup 1.0000x reference)
"""Varlen causal GQA attention on 8 trn2 NeuronCores, head-parallel sharding.

Each core takes 2 of the 16 query heads plus their shared GQA KV head and
processes all sequences. Flash-attention style blocks of 128 tokens; scores
are computed transposed ([k, q] layout) so the AV matmul needs no transposes,
and V is augmented with a ones-column so the softmax denominator falls out of
the same PSUM accumulation. exp() without max-subtraction is exact here:
scores are O(5) and the reference's -10000 masking underflows to 0 in fp32.
"""

import sys

sys.path.insert(0, "/opt/trn_rl_repo")

import numpy as np
import ml_dtypes

import concourse.bass as bass
import concourse.mybir as mybir
import concourse.tile as tile
import concourse.bacc as bacc
from concourse.bass_utils import run_bass_kernel_spmd

N_CORES = 8
H = 16
HKV = 4
D = 64
HEADS_PER_CORE = H // N_CORES  # 2
BLK = 128
GRP = 512  # q tokens per group (4 blocks, one PSUM bank wide)
SCALE = 0.125  # 1/sqrt(64)

BF16 = mybir.dt.bfloat16
F32 = mybir.dt.float32
np_bf16 = ml_dtypes.bfloat16

_cache = {}


def _build(lens):
    """Build the SPMD Bass program for per-seq padded lengths (multiples of 128)."""
    Ts = [L // BLK for L in lens]
    starts = np.concatenate([[0], np.cumsum(lens)])
    N = int(starts[-1])  # padded total tokens
    NB = N // BLK  # global 128-token blocks
    assert NB % 2 == 0 or True

    nc = bacc.Bacc("TRN2", target_bir_lowering=False, debug=False,
                   num_devices=N_CORES)

    qT_d = nc.dram_tensor("qT", [HEADS_PER_CORE * D, N], BF16, kind="ExternalInput")
    kT_d = nc.dram_tensor("kT", [D, N], BF16, kind="ExternalInput")
    v_d = nc.dram_tensor("v", [N, D], BF16, kind="ExternalInput")
    mtri_d = nc.dram_tensor("mtri", [BLK, BLK], BF16, kind="ExternalInput")
    oT_d = nc.dram_tensor("oT", [HEADS_PER_CORE, D + 1, N], F32,
                          kind="ExternalOutput")

    with tile.TileContext(nc) as tc:
        with (
            tc.tile_pool(name="const", bufs=1) as const,
            tc.tile_pool(name="pt_pool", bufs=3) as pt_pool,
            tc.tile_pool(name="ot_pool", bufs=3) as ot_pool,
            tc.tile_pool(name="ps_st", bufs=3, space="PSUM") as ps_st,
            tc.tile_pool(name="ps_ot", bufs=2, space="PSUM") as ps_ot,
        ):
            qt = const.tile([HEADS_PER_CORE * D, N], BF16)
            # kT duplicated into both partition halves so head h's QK matmul
            # finds lhsT and rhs at the same base partition 64*h
            kt = const.tile([2 * D, N], BF16)
            vp = const.tile([BLK, NB * (D + 1)], BF16)
            mtri = const.tile([BLK, BLK], BF16)
            nc.sync.dma_start(qt[:], qT_d[:])
            nc.sync.dma_start(kt[0:D, :], kT_d[:])
            nc.sync.dma_start(kt[D:2 * D, :], kT_d[:])
            nc.sync.dma_start(mtri[:], mtri_d[:])
            nc.vector.memset(vp[:], 1.0)
            # v rows (jg*128 + p) land at vp[p, jg*65 : jg*65+64]
            v_view = v_d.rearrange("(j p) d -> p j d", p=BLK)
            vp_view = vp.rearrange("p (j e) -> p j e", e=D + 1)
            nc.sync.dma_start(vp_view[:, :, 0:D], v_view[:])

            def kt_ap(jg, h):
                return kt[D * h:D * h + D, BLK * jg:BLK * jg + BLK]

            for h in range(HEADS_PER_CORE):
                for b, L in enumerate(lens):
                    T = Ts[b]
                    s0 = int(starts[b])
                    for g in range((T + 3) // 4):
                        q0 = s0 + GRP * g
                        W = min(GRP, L - GRP * g)
                        jmax = min(4 * g + 3, T - 1)
                        po = ps_ot.tile([D + 1, GRP], F32)
                        for j in range(jmax + 1):
                            jg = (s0 // BLK) + j
                            c_off = max(0, BLK * (j - 4 * g))
                            Wp = W - c_off
                            st = ps_st.tile([BLK, GRP], F32)
                            nc.tensor.matmul(
                                st[:, :Wp],
                                kt_ap(jg, h),
                                qt[D * h:D * h + D, q0 + c_off:q0 + W],
                                start=True, stop=True,
                            )
                            pt = pt_pool.tile([BLK, GRP], BF16)
                            nc.scalar.activation(
                                pt[:, :Wp], st[:, :Wp],
                                mybir.ActivationFunctionType.Exp, scale=SCALE,
                            )
                            if j >= 4 * g:
                                # triangular boundary: first 128 cols of slice
                                nc.vector.tensor_tensor(
                                    pt[:, 0:BLK], pt[:, 0:BLK], mtri[:],
                                    mybir.AluOpType.mult,
                                )
                            nc.tensor.matmul(
                                po[:, c_off:W],
                                vp[:, jg * (D + 1):(jg + 1) * (D + 1)],
                                pt[:, :Wp],
                                start=(j == 0), stop=(j == jmax),
                            )
                        ot = ot_pool.tile([D + 1, GRP], F32)
                        nc.scalar.copy(ot[:, :W], po[:, :W])
                        nc.sync.dma_start(oT_d[h, :, q0:q0 + W], ot[:, :W])

    nc.compile()
    return nc


def kernel(q, kv, cu_seqlens, max_seqlen=None):
    q = np.asarray(q, dtype=np.float32)
    kv = np.asarray(kv, dtype=np.float32)
    cu = np.asarray(cu_seqlens).astype(np.int64)
    total, Hq, Dq = q.shape
    lens_raw = (cu[1:] - cu[:-1]).tolist()
    lens = [max(BLK, -(-int(L) // BLK) * BLK) for L in lens_raw]
    key = tuple(lens)
    if key not in _cache:
        _cache[key] = _build(lens)
    nc = _cache[key]

    starts_p = np.concatenate([[0], np.cumsum(lens)])
    N = int(starts_p[-1])

    # pad ragged sequences into the 128-aligned packed layout
    qp = np.zeros((N, H, D), np.float32)
    kp = np.zeros((N, HKV, D), np.float32)
    vfull = np.zeros((N, HKV, D), np.float32)
    for b in range(len(lens)):
        s, e = int(cu[b]), int(cu[b + 1])
        d0 = int(starts_p[b])
        qp[d0:d0 + e - s] = q[s:e]
        kp[d0:d0 + e - s] = kv[s:e, 0]
        vfull[d0:d0 + e - s] = kv[s:e, 1]

    mtri = np.triu(np.ones((BLK, BLK), np_bf16))  # [p, x] = x >= p

    in_maps = []
    for c in range(N_CORES):
        hq = [HEADS_PER_CORE * c + i for i in range(HEADS_PER_CORE)]
        ckv = hq[0] // (H // HKV)
        qT = np.ascontiguousarray(
            qp[:, hq, :].transpose(1, 2, 0).reshape(HEADS_PER_CORE * D, N)
        ).astype(np_bf16)
        kT = np.ascontiguousarray(kp[:, ckv, :].T).astype(np_bf16)  # [64, N]
        vc = np.ascontiguousarray(vfull[:, ckv, :]).astype(np_bf16)
        in_maps.append({"qT": qT, "kT": kT, "v": vc, "mtri": mtri})

    res = run_bass_kernel_spmd(nc, in_maps, core_ids=list(range(N_CORES)))
    global last_results
    last_results = res

    out = np.empty((total, H, D), np.float32)
    for c in range(N_CORES):
        oT = res.results[c]["oT"]  # [2, 65, N]
        o = oT[:, :D, :] / oT[:, D:D + 1, :]  # [2, 64, N]
        o = o.transpose(2, 0, 1)  # [N, 2, 64]
        for b in range(len(lens)):
            s, e = int(cu[b]), int(cu[b + 1])
            d0 = int(starts_p[b])
            out[s:e, HEADS_PER_CORE * c:HEADS_PER_CORE * (c + 1), :] = \
                o[d0:d0 + e - s]
    return out


# revision 21
# speedup vs baseline: 1.3635x; 1.3635x over previous
"""Varlen causal GQA attention on 8 trn2 NeuronCores, head-parallel sharding.

Each core takes 2 of the 16 query heads plus their shared GQA KV head and
processes all sequences. Flash-attention style blocks of 128 tokens; scores
are computed transposed ([k, q] layout) so the AV matmul needs no transposes,
and V is augmented with a ones-column so the softmax denominator falls out of
the same PSUM accumulation. exp() without max-subtraction is exact here:
scores are O(5) and the reference's -10000 masking underflows to 0 in fp32.

Layout notes:
 - qt SBUF [128, N]: head h occupies partitions 64h..64h+63 (D=64 rows); kt is
   duplicated into both partition halves so each head's QK matmul sees lhsT
   and rhs at the same base partition -> PE row-group packing lets the two
   heads' QK matmuls overlap in the array.
 - Diagonal (i==j) score blocks for the whole head are packed 4-per-PSUM-bank,
   exp'd with one ACT call per bank and causal-masked with one broadcast
   tensor_tensor per bank into a persistent ptd buffer.
 - Off-diagonal key blocks are processed 2 per PSUM chunk with a single
   3D-access-pattern exp call.
"""

import sys

sys.path.insert(0, "/opt/trn_rl_repo")

import os
import numpy as np
import ml_dtypes

DEBUG_PTD = os.environ.get("DEBUG_PTD", "0") == "1"

import concourse.bass as bass
import concourse.mybir as mybir
import concourse.tile as tile
import concourse.bacc as bacc
from concourse.bass_utils import run_bass_kernel_spmd

N_CORES = 8
H = 16
HKV = 4
D = 64
HPC = H // N_CORES  # heads per core
BLK = 128
GRP = 512  # q tokens per group (4 blocks, one PSUM bank wide)
SCALE = 0.125  # 1/sqrt(64)

BF16 = mybir.dt.bfloat16
F32 = mybir.dt.float32
np_bf16 = ml_dtypes.bfloat16
Exp = mybir.ActivationFunctionType.Exp

_cache = {}


def _build(lens):
    """Build the SPMD Bass program for per-seq padded lengths (multiples of 128)."""
    lens = [int(L) for L in lens]
    Ts = [L // BLK for L in lens]
    starts = [0]
    for L in lens:
        starts.append(starts[-1] + L)
    N = starts[-1]
    NB = N // BLK

    nc = bacc.Bacc("TRN2", target_bir_lowering=False, debug=False,
                   num_devices=N_CORES)

    qT_d = nc.dram_tensor("qT", [HPC * D, N], BF16, kind="ExternalInput")
    kT_d = nc.dram_tensor("kT", [D, N], BF16, kind="ExternalInput")
    v_d = nc.dram_tensor("v", [N, D + 1], BF16, kind="ExternalInput")
    mtri_d = nc.dram_tensor("mtri", [BLK, 4 * BLK], BF16, kind="ExternalInput")
    oT_d = nc.dram_tensor("oT", [HPC, D + 1, N], F32, kind="ExternalOutput")

    with tile.TileContext(nc) as tc:
        with (
            tc.tile_pool(name="const", bufs=1) as const,
            tc.tile_pool(name="pt_pool", bufs=3) as pt_pool,
            tc.tile_pool(name="ot_pool", bufs=3) as ot_pool,
            tc.tile_pool(name="ps_o", bufs=3, space="PSUM") as ps_o,
            tc.tile_pool(name="ps_po", bufs=1, space="PSUM") as ps_po,
        ):
            qt = const.tile([HPC * D, N], BF16)
            kt = const.tile([2 * D, N], BF16)
            vp = const.tile([BLK, NB * (D + 1)], BF16)
            mtri = const.tile([BLK, 4 * BLK], BF16)
            nc.sync.dma_start(qt[:], qT_d[:])
            nc.sync.dma_start(kt[0:D, :], kT_d[:])
            nc.sync.dma_start(kt[D:2 * D, :], kT_d[:])
            nc.sync.dma_start(mtri[:], mtri_d[:])
            vp_view = vp.rearrange("p (j e) -> p j e", e=D + 1)
            nc.sync.dma_start(vp_view[:], v_d.rearrange("(j p) e -> p j e", p=BLK))

            def kt_ap(jg, h):
                return kt[D * h:D * h + D, BLK * jg:BLK * jg + BLK]

            def vp_ap(jg):
                return vp[:, jg * (D + 1):(jg + 1) * (D + 1)]

            # ---- phase A: all diagonal-region wedges, batched by width ----
            # wedge (b, g, r): k-block jg = q0/128 + r, q-cols [q0+128r, q0+W)
            wedges = []  # (Wc, jg, qs, width) in deterministic order
            for b, Lb in enumerate(lens):
                s0 = starts[b]
                T = Ts[b]
                for g in range((T + 3) // 4):
                    q0 = s0 + GRP * g
                    W = min(GRP, Lb - GRP * g)
                    for r in range(W // BLK):
                        wedges.append((q0 // BLK + r, q0 + BLK * r, W - BLK * r))
            # assign ptw offsets grouped by width class
            SLOTS = {512: 2, 384: 2, 256: 4, 128: 8}
            STRIDE = {512: 512, 384: 512, 256: 256, 128: 128}
            by_w = {}
            for jg, qs, wd in wedges:
                by_w.setdefault(wd, []).append((jg, qs))
            ptw_off = {}
            off = 0
            chunks = []  # (width, [(jg, qs, off)...])
            for wd in sorted(by_w, reverse=True):
                lst = by_w[wd]
                s = SLOTS[wd]
                for c0 in range(0, len(lst), s):
                    grp = []
                    for jg, qs in lst[c0:c0 + s]:
                        ptw_off[jg] = off
                        grp.append((jg, qs, off))
                        off += wd
                    chunks.append((wd, grp))
            ptw_sz = off
            ptw = const.tile([BLK, HPC * ptw_sz], BF16)
            ptw_h = [ptw[:, h * ptw_sz:(h + 1) * ptw_sz] for h in range(HPC)]

            for wd, grp in chunks:
                stride = STRIDE[wd]
                for h in range(HPC):
                    psc = ps_o.tile([BLK, 1024], F32, name="psc")
                    for t, (jg, qs, _o) in enumerate(grp):
                        nc.tensor.matmul(
                            psc[:, stride * t:stride * t + wd],
                            kt_ap(jg, h),
                            qt[D * h:D * h + D, qs:qs + wd],
                            start=True, stop=True,
                        )
                    o0 = grp[0][2]
                    n = len(grp)
                    dst = ptw_h[h][:, o0:o0 + n * wd]
                    if stride == wd:
                        nc.scalar.activation(dst, psc[:, :n * wd], Exp,
                                             scale=SCALE)
                    else:
                        src3 = psc.rearrange("p (t c) -> p t c", c=stride)
                        nc.scalar.activation(
                            dst.rearrange("p (t c) -> p t c", c=wd),
                            src3[:, :n, :wd], Exp, scale=SCALE)
                    for _jg, _qs, o in grp:
                        tri = ptw_h[h][:, o:o + BLK]
                        nc.vector.tensor_tensor(tri, tri, mtri[:, :BLK],
                                                mybir.AluOpType.mult)

            # ---- main: per (seq, q-group): wedge AVs then off-diag chunks ----
            for b, Lb in enumerate(lens):
                T = Ts[b]
                s0 = starts[b]
                for g in range((T + 3) // 4):
                    q0 = s0 + GRP * g
                    W = min(GRP, Lb - GRP * g)
                    nblk = W // BLK
                    po = [ps_po.tile([D + 1, GRP], F32, name=f"po{h}")
                          for h in range(HPC)]
                    for h in range(HPC):
                        for r in range(nblk):
                            jg = q0 // BLK + r
                            o = ptw_off[jg]
                            wd = W - BLK * r
                            nc.tensor.matmul(
                                po[h][:, BLK * r:BLK * r + wd],
                                vp_ap(jg),
                                ptw_h[h][:, o:o + wd],
                                start=(r == 0),
                                stop=(g == 0 and r == nblk - 1),
                            )
                    for c in range(2 * g):
                        js = [2 * c, 2 * c + 1]
                        for h in range(HPC):
                            ps2 = ps_o.tile([BLK, 1024], F32, name="psc")
                            for t, j in enumerate(js):
                                jg = s0 // BLK + j
                                nc.tensor.matmul(
                                    ps2[:, 512 * t:512 * t + W],
                                    kt_ap(jg, h),
                                    qt[D * h:D * h + D, q0:q0 + W],
                                    start=True, stop=True,
                                )
                            pto = pt_pool.tile([BLK, 1024], BF16)
                            if W == GRP:
                                nc.scalar.activation(pto[:], ps2[:], Exp,
                                                     scale=SCALE)
                            else:
                                nc.scalar.activation(
                                    pto.rearrange("p (t c) -> p t c", c=512)[:, :, :W],
                                    ps2.rearrange("p (t c) -> p t c", c=512)[:, :, :W],
                                    Exp, scale=SCALE)
                            for t, j in enumerate(js):
                                jg = s0 // BLK + j
                                nc.tensor.matmul(
                                    po[h][:, :W],
                                    vp_ap(jg),
                                    pto[:, 512 * t:512 * t + W],
                                    start=False,
                                    stop=(c == 2 * g - 1 and t == 1),
                                )
                    for h in range(HPC):
                        ot = ot_pool.tile([D + 1, GRP], F32, name=f"ot{h}")
                        nc.vector.tensor_copy(ot[:, :W], po[h][:, :W])
                        nc.sync.dma_start(oT_d[h, :, q0:q0 + W], ot[:, :W])

    nc.compile()
    return nc


def kernel(q, kv, cu_seqlens, max_seqlen=None):
    q = np.asarray(q, dtype=np.float32)
    kv = np.asarray(kv, dtype=np.float32)
    cu = np.asarray(cu_seqlens).astype(np.int64)
    total = q.shape[0]
    lens_raw = (cu[1:] - cu[:-1]).tolist()
    lens = [max(BLK, -(-int(L) // BLK) * BLK) for L in lens_raw]
    key = tuple(lens)
    if key not in _cache:
        _cache[key] = _build(lens)
    nc = _cache[key]

    starts_p = np.concatenate([[0], np.cumsum(lens)])
    N = int(starts_p[-1])

    # pad ragged sequences into the 128-aligned packed layout
    qp = np.zeros((N, H, D), np.float32)
    kp = np.zeros((N, HKV, D), np.float32)
    v65 = np.zeros((N, HKV, D + 1), np.float32)
    for b in range(len(lens)):
        s, e = int(cu[b]), int(cu[b + 1])
        d0 = int(starts_p[b])
        qp[d0:d0 + e - s] = q[s:e]
        kp[d0:d0 + e - s] = kv[s:e, 0]
        v65[d0:d0 + e - s, :, :D] = kv[s:e, 1]
    v65[:, :, D] = 1.0

    # [p, x] = x >= p, tiled 4x along free dim for packed diagonal chunks
    mtri = np.tile(np.triu(np.ones((BLK, BLK), np_bf16)), (1, 4))

    in_maps = []
    for c in range(N_CORES):
        hq = [HPC * c + i for i in range(HPC)]
        ckv = hq[0] // (H // HKV)
        qT = np.ascontiguousarray(
            qp[:, hq, :].transpose(1, 2, 0).reshape(HPC * D, N)
        ).astype(np_bf16)
        kT = np.ascontiguousarray(kp[:, ckv, :].T).astype(np_bf16)  # [64, N]
        vc = np.ascontiguousarray(v65[:, ckv, :]).astype(np_bf16)
        in_maps.append({"qT": qT, "kT": kT, "v": vc, "mtri": mtri})

    res = run_bass_kernel_spmd(nc, in_maps, core_ids=list(range(N_CORES)))
    global last_results
    last_results = res

    out = np.empty((total, H, D), np.float32)
    for c in range(N_CORES):
        oT = res.results[c]["oT"]  # [HPC, 65, N]
        o = oT[:, :D, :] / oT[:, D:D + 1, :]  # [HPC, 64, N]
        o = o.transpose(2, 0, 1)  # [N, HPC, 64]
        for b in range(len(lens)):
            s, e = int(cu[b]), int(cu[b + 1])
            d0 = int(starts_p[b])
            out[s:e, HPC * c:HPC * (c + 1), :] = o[d0:d0 + e - s]
    return out
